# revision 31
# baseline (speedup 1.0000x reference)
"""Trainium2 Bass kernel for NeuralECMModel (gnn_message_passing).

Computation (per node n):
  ent  = entity_emb @ Wp.T + bp                                  [N,50]
  node = einsum('ni,oij,nj->no', q, Wbil, ent) + bbil            [N,50]
  wtext= sum_k scores[n,k]*nbr[n,k,:] + scores[n,63]*node[n,:]   [N,50]
  agg  = wtext @ Wg.T                                            [N,50]
  feats= elu(agg + g_bias)
  out  = feats @ Wr.T + br                                       [N,1]

Sharding: pure data parallel over nodes, N=20000 -> 2500 nodes/core x 8.

Device mapping per 128-node tile:
  - PE: entity projection (host-transposed entity as stationary operand),
    bilinear stage 1 (U = qT.T @ W2 with W2[i,(o,j)]=Wbil[o,i,j]),
    wtext transpose, Wg matmul, Wr head matmul.
  - DVE: neighbor score-multiply + k-reduce, bilinear stage 2
    (U*ent j-reduce), small fixups, ELU tail assists.
  - GPSIMD: score broadcast-expansion (1-input, line rate).
  - ACT: PSUM->SBUF moves, exp/relu for ELU.
"""

import numpy as np
import ml_dtypes

import concourse.bass as bass
import concourse.bacc as bacc
import concourse.tile as tile
import concourse.mybir as mybir
from concourse.bass_utils import run_bass_kernel_spmd
from concourse.masks import make_identity

F32 = mybir.dt.float32
BF16 = mybir.dt.bfloat16
AX = mybir.AxisListType
OP = mybir.AluOpType
AF = mybir.ActivationFunctionType

N_CORES = 8
N = 20000
NLOC = N // N_CORES  # 2500
K = 63
D = 50
E = 300
OJ = D * D  # 2500
P = 128
N_TILES = (NLOC + P - 1) // P  # 20
NBR_DT = BF16  # dtype for neighbor embeddings + scores on device
PRODB_BUFS = 3
PRODN_BUFS = 3
PSU_BUFS = 2

_CACHE = {}


def _np_dt(dt):
    return ml_dtypes.bfloat16 if dt == BF16 else np.float32


def build_program(br_val: float, nbr_pl=0, bil_pl=0, repeat=1, ds=42, os_=46, smalls_pool=True):
    nc = bacc.Bacc("TRN2", debug=False, num_devices=N_CORES)
    nbr_dt = NBR_DT

    # Per-core inputs (host pre-laid-out)
    t_nbr = nc.dram_tensor("nbr", [NLOC, K * D], nbr_dt, kind="ExternalInput")
    t_sc = nc.dram_tensor("scores", [NLOC, K + 1], nbr_dt, kind="ExternalInput")
    t_sc63 = nc.dram_tensor("s63", [NLOC, 1], F32, kind="ExternalInput")
    t_qT = nc.dram_tensor("qT", [D, NLOC], BF16, kind="ExternalInput")
    t_entT0 = nc.dram_tensor("entT0", [128, NLOC], F32, kind="ExternalInput")
    t_entT1 = nc.dram_tensor("entT1", [128, NLOC], F32, kind="ExternalInput")
    t_entT2 = nc.dram_tensor("entT2", [45, NLOC], F32, kind="ExternalInput")
    # Replicated weights
    t_W2 = nc.dram_tensor("W2", [D, OJ], BF16, kind="ExternalInput")
    t_WpT0 = nc.dram_tensor("WpT0", [128, D], F32, kind="ExternalInput")
    t_WpT1 = nc.dram_tensor("WpT1", [128, D], F32, kind="ExternalInput")
    t_WpT2 = nc.dram_tensor("WpT2", [45, D], F32, kind="ExternalInput")
    t_WgT = nc.dram_tensor("WgT", [D, D], F32, kind="ExternalInput")
    t_WrT = nc.dram_tensor("WrT", [D, 1], F32, kind="ExternalInput")
    t_bbil = nc.dram_tensor("bbil_rep", [P, D], F32, kind="ExternalInput")
    t_gb = nc.dram_tensor("gbias_col", [D, 1], F32, kind="ExternalInput")
    t_out = nc.dram_tensor("out", [NLOC, 1], F32, kind="ExternalOutput")

    with tile.TileContext(nc) as tc:
        with (
            tc.tile_pool(name="res", bufs=1) as res,
            tc.tile_pool(name="nbrp", bufs=4) as nbrp,
            tc.tile_pool(name="scp", bufs=4) as scp,
            tc.tile_pool(name="prodn", bufs=PRODN_BUFS) as prodnp,
            tc.tile_pool(name="prodb", bufs=PRODB_BUFS) as prodbp,
            tc.tile_pool(name="small", bufs=4) as small,
            tc.tile_pool(name="tail", bufs=4) as tailp,
            tc.tile_pool(name="outp", bufs=4) as outp,
            tc.tile_pool(name="ps_ent", bufs=1, space="PSUM") as ps_ent,
            tc.tile_pool(name="ps_u", bufs=PSU_BUFS, space="PSUM") as ps_u,
            tc.tile_pool(name="ps_t", bufs=1, space="PSUM") as ps_t,
            tc.tile_pool(name="ps_agg", bufs=2, space="PSUM") as ps_agg,
            tc.tile_pool(name="ps_o", bufs=1, space="PSUM") as ps_o,
        ):
            # ---- resident loads (once) ----
            qT_sb = res.tile([D, NLOC], BF16)
            nc.sync.dma_start(out=qT_sb, in_=t_qT[:])
            entT_sb = [
                res.tile([128, NLOC], F32, tag="entT0", name="entT0_sb"),
                res.tile([128, NLOC], F32, tag="entT1", name="entT1_sb"),
                res.tile([45, NLOC], F32, tag="entT2", name="entT2_sb"),
            ]
            for sb, t in zip(entT_sb, (t_entT0, t_entT1, t_entT2)):
                nc.sync.dma_start(out=sb, in_=t[:])
            W2_sb = res.tile([D, OJ], BF16)
            nc.sync.dma_start(out=W2_sb, in_=t_W2[:])
            WpT_sb = [
                res.tile([128, D], F32, tag="WpT0", name="WpT0_sb"),
                res.tile([128, D], F32, tag="WpT1", name="WpT1_sb"),
                res.tile([45, D], F32, tag="WpT2", name="WpT2_sb"),
            ]
            for sb, t in zip(WpT_sb, (t_WpT0, t_WpT1, t_WpT2)):
                nc.sync.dma_start(out=sb, in_=t[:])
            WgT_sb = res.tile([D, D], F32)
            nc.sync.dma_start(out=WgT_sb, in_=t_WgT[:])
            WrT_sb = res.tile([D, 1], F32)
            nc.sync.dma_start(out=WrT_sb, in_=t_WrT[:])
            bbil_sb = res.tile([P, D], F32)
            nc.sync.dma_start(out=bbil_sb, in_=t_bbil[:])
            gb_sb = res.tile([D, 1], F32)
            nc.sync.dma_start(out=gb_sb, in_=t_gb[:])
            ident_sb = res.tile([P, P], F32)
            make_identity(nc, ident_sb)
            zeros_sb = res.tile([D, P], F32)
            nc.vector.memset(zeros_sb, 0.0)

            NCH = 5  # bilinear oj chunks
            CW = OJ // NCH  # 500
            O_PER = CW // D  # 10 o's per chunk

            def tree_reduce_blocks(buf, rows, nblk, width, out_ap):
                """Sum `nblk` contiguous blocks of `width` elems (axis -1 of
                buf[:rows]) via pairwise adds; final add writes out_ap (f32)."""
                cur = nblk
                while cur > 2:
                    lo = (cur + 1) // 2
                    hi = cur - lo
                    nc.vector.tensor_add(
                        buf[:rows, 0 : hi * width],
                        buf[:rows, 0 : hi * width],
                        buf[:rows, lo * width : (lo + hi) * width],
                    )
                    cur = lo
                if cur == 2:
                    nc.vector.tensor_add(
                        out_ap,
                        buf[:rows, 0:width],
                        buf[:rows, width : 2 * width],
                    )
                else:
                    nc.vector.tensor_copy(out_ap, buf[:rows, 0:width])

            def tree_reduce_inner(eng, v, out_ap, width):
                """v: [rows, nblk, width] view; sum inner axis into out_ap
                (f32 [rows, nblk]) on engine `eng`. Splits keep 4B align."""
                w = width
                while w > 2:
                    lo = (w // 2 + 1) // 2 * 2  # even split point >= w/2
                    hi = w - lo
                    eng.tensor_add(
                        v[:, :, 0:hi], v[:, :, 0:hi], v[:, :, lo : lo + hi]
                    )
                    w = lo
                eng.tensor_add(
                    out_ap.unsqueeze(2), v[:, :, 0:1], v[:, :, 1:2]
                )

            import contextlib

            rep_ctx = (
                tc.For_i(0, repeat, 1) if repeat > 1 else contextlib.nullcontext()
            )
            with rep_ctx:
              for it in range(N_TILES):
                r0 = it * P
                rows = min(P, NLOC - r0)
                rs = slice(r0, r0 + rows)

                nbr_t = nbrp.tile([P, K * D], nbr_dt)
                nc.sync.dma_start(out=nbr_t[:rows], in_=t_nbr[rs, :])
                sc_t = scp.tile([P, K + 1], nbr_dt)
                nc.sync.dma_start(out=sc_t[:rows], in_=t_sc[rs, :])
                s63_t = scp.tile([P, 1], F32, tag="s63")
                nc.sync.dma_start(out=s63_t[:rows], in_=t_sc63[rs, :])

                # --- entity projection on PE: ent[n,j] ---
                ent_ps = ps_ent.tile([P, D], F32)
                for c in range(3):
                    nc.tensor.matmul(
                        ent_ps[:rows],
                        entT_sb[c][:, rs],
                        WpT_sb[c],
                        start=(c == 0),
                        stop=(c == 2),
                    )
                ent_sb = small.tile([P, D], BF16, tag="ent")
                nc.scalar.copy(out=ent_sb[:rows], in_=ent_ps[:rows])

                # --- neighbor stage (nbr is d-major: nbr[n, d*K+k]) ---
                # d-range [0, ds) on DVE, [ds, D) on GPSIMD
                prodn = prodnp.tile([P, K * D], nbr_dt)
                pn = prodn[:rows].rearrange("p (d k) -> p d k", d=D)
                nb = nbr_t[:rows].rearrange("p (d k) -> p d k", d=D)
                scb = sc_t[:rows, 0:K].unsqueeze(1).broadcast_to([rows, D, K])
                wnbr = small.tile([P, D], F32, tag="wnbr")
                if ds > 0:
                    nc.vector.tensor_mul(
                        pn[:, 0:ds], nb[:, 0:ds], scb[:, 0:ds]
                    )
                    tree_reduce_inner(
                        nc.vector, pn[:, 0:ds], wnbr[:rows, 0:ds], K
                    )
                if ds < D:
                    nc.gpsimd.tensor_mul(
                        pn[:, ds:D], nb[:, ds:D], scb[:, ds:D]
                    )
                    tree_reduce_inner(
                        nc.gpsimd, pn[:, ds:D], wnbr[:rows, ds:D], K
                    )

                # --- bilinear stage ---
                prodb = prodbp.tile([P, OJ], BF16)
                usb = prodbp.tile([P, OJ], BF16, tag="usb")
                for c in range(NCH):
                    u_ps = ps_u.tile([P, CW], F32)
                    nc.tensor.matmul(
                        u_ps[:rows],
                        qT_sb[:, rs],
                        W2_sb[:, c * CW : (c + 1) * CW],
                        start=True,
                        stop=True,
                    )
                    nc.scalar.copy(
                        out=usb[:rows, c * CW : (c + 1) * CW], in_=u_ps[:rows]
                    )
                pb = prodb[:rows].rearrange("p (o j) -> p o j", o=D)
                ub = usb[:rows].rearrange("p (o j) -> p o j", o=D)
                eb = ent_sb[:rows].unsqueeze(1).broadcast_to([rows, D, D])
                noderaw = small.tile([P, D], F32, tag="noderaw")
                if os_ > 0:
                    nc.vector.tensor_mul(pb[:, 0:os_], ub[:, 0:os_], eb[:, 0:os_])
                    tree_reduce_inner(
                        nc.vector, pb[:, 0:os_], noderaw[:rows, 0:os_], D
                    )
                if os_ < D:
                    nc.gpsimd.tensor_mul(pb[:, os_:D], ub[:, os_:D], eb[:, os_:D])
                    tree_reduce_inner(
                        nc.gpsimd, pb[:, os_:D], noderaw[:rows, os_:D], D
                    )
                smeng = nc.gpsimd if smalls_pool else nc.vector
                nodeb = small.tile([P, D], F32, tag="nodeb")
                smeng.tensor_add(nodeb[:rows], noderaw[:rows], bbil_sb[:rows])

                # wtext = nodeb * s63 + wnbr
                wtext = small.tile([P, D], F32, tag="wtext")
                nc.vector.scalar_tensor_tensor(
                    out=wtext[:rows],
                    in0=nodeb[:rows],
                    scalar=s63_t[:rows],
                    in1=wnbr[:rows],
                    op0=OP.mult,
                    op1=OP.add,
                )

                # --- tail: agg = wtext @ Wg.T; feats=elu(agg+gb); out=feats@Wr.T+br
                wtT_ps = ps_t.tile([D, P], F32)
                nc.tensor.transpose(
                    wtT_ps[:, :rows], wtext[:rows], ident_sb[:rows, :rows]
                )
                wtT_sb = tailp.tile([D, P], F32, tag="wtT")
                nc.scalar.copy(out=wtT_sb[:, :rows], in_=wtT_ps[:, :rows])
                agg_ps = ps_agg.tile([D, P], F32)
                nc.tensor.matmul(
                    agg_ps[:, :rows], WgT_sb, wtT_sb[:, :rows], start=True, stop=True
                )
                e_sb = tailp.tile([D, P], F32, tag="e")
                nc.scalar.activation(
                    out=e_sb[:, :rows], in_=agg_ps[:, :rows], func=AF.Exp, bias=gb_sb
                )
                r_sb = tailp.tile([D, P], F32, tag="r")
                nc.scalar.activation(
                    out=r_sb[:, :rows], in_=agg_ps[:, :rows], func=AF.Relu, bias=gb_sb
                )
                feats = tailp.tile([D, P], F32, tag="feats")
                nc.vector.scalar_tensor_tensor(
                    out=feats[:, :rows],
                    in0=e_sb[:, :rows],
                    scalar=1.0,
                    in1=zeros_sb[:, :rows],
                    op0=OP.subtract,
                    op1=OP.min,
                )
                smeng.tensor_add(feats[:, :rows], feats[:, :rows], r_sb[:, :rows])
                out_ps = ps_o.tile([1, P], F32)
                nc.tensor.matmul(
                    out_ps[:, :rows], WrT_sb, feats[:, :rows], start=True, stop=True
                )
                out_sb = outp.tile([1, P], F32)
                nc.scalar.activation(
                    out=out_sb[:, :rows],
                    in_=out_ps[:, :rows],
                    func=AF.Identity,
                    bias=br_val,
                )
                nc.sync.dma_start(
                    out=t_out[rs, :].transpose([1, 0]), in_=out_sb[:, :rows]
                )

    nc.finalize()
    return nc


def kernel(
    query_emb,
    entity_emb,
    neighbor_embs,
    neighbor_scores,
    Wp,
    bp,
    Wbil,
    bbil,
    Wg,
    g_bias,
    Wr,
    br,
):
    nbr_np = _np_dt(NBR_DT)
    br_val = float(np.asarray(br).reshape(-1)[0])

    if "nc" not in _CACHE:
        _CACHE["nc"] = build_program(br_val)
    nc = _CACHE["nc"]

    # ---- shared weight prep ----
    # W2[i, o*D+j] = Wbil[o, i, j]
    W2 = np.ascontiguousarray(
        np.asarray(Wbil, np.float32).transpose(1, 0, 2).reshape(D, OJ)
    ).astype(ml_dtypes.bfloat16)
    WpT_aug = np.concatenate(
        [np.asarray(Wp, np.float32).T, np.asarray(bp, np.float32)[None, :]], axis=0
    )  # [301, 50]
    WpT_chunks = [
        np.ascontiguousarray(WpT_aug[0:128]),
        np.ascontiguousarray(WpT_aug[128:256]),
        np.ascontiguousarray(WpT_aug[256:301]),
    ]
    WgT = np.ascontiguousarray(np.asarray(Wg, np.float32).T)
    WrT = np.ascontiguousarray(np.asarray(Wr, np.float32).T)
    bbil_rep = np.ascontiguousarray(
        np.tile(np.asarray(bbil, np.float32)[None, :], (P, 1))
    )
    gb_col = np.ascontiguousarray(np.asarray(g_bias, np.float32)[:, None])

    q = np.asarray(query_emb, np.float32)
    ent = np.asarray(entity_emb, np.float32)
    nbr = np.asarray(neighbor_embs, np.float32)
    sc = np.asarray(neighbor_scores, np.float32)

    in_maps = []
    for c in range(N_CORES):
        s = slice(c * NLOC, (c + 1) * NLOC)
        ent_aug = np.concatenate(
            [ent[s], np.ones((NLOC, 1), np.float32)], axis=1
        ).T  # [301, NLOC]
        ent_aug = np.ascontiguousarray(ent_aug)
        in_maps.append(
            {
                "nbr": np.ascontiguousarray(
                    nbr[s].transpose(0, 2, 1).reshape(NLOC, K * D)
                ).astype(nbr_np),
                "scores": np.ascontiguousarray(sc[s]).astype(nbr_np),
                "s63": np.ascontiguousarray(sc[s, K : K + 1]),
                "qT": np.ascontiguousarray(q[s].T).astype(ml_dtypes.bfloat16),
                "entT0": np.ascontiguousarray(ent_aug[0:128]),
                "entT1": np.ascontiguousarray(ent_aug[128:256]),
                "entT2": np.ascontiguousarray(ent_aug[256:301]),
                "W2": W2,
                "WpT0": WpT_chunks[0],
                "WpT1": WpT_chunks[1],
                "WpT2": WpT_chunks[2],
                "WgT": WgT,
                "WrT": WrT,
                "bbil_rep": bbil_rep,
                "gbias_col": gb_col,
            }
        )

    _CACHE["last_in_maps"] = in_maps
    res = run_bass_kernel_spmd(nc, in_maps, core_ids=list(range(N_CORES)))
    out = np.concatenate([res.results[c]["out"] for c in range(N_CORES)], axis=0)
    return out.astype(np.float32)
